# revision 15
# baseline (speedup 1.0000x reference)
"""GAT (PyG GATConv + Linear) on 8 Trainium2 NeuronCores.

Strategy (edge-parallel / 1D node partition, per the sharding hint, adapted to
this environment):
  - Nodes are range-partitioned over the 8 cores by destination id
    (12500 dst nodes per core); each core holds its edge shard.
  - Kernel-1 (device): F = [h(16) | a_src(8) | a_dst(8)] = dense projections
    of x (x @ W and the per-head attention dots), node-sharded across cores.
  - Host: pure index shuffling -- the per-edge join F[src_e] is materialized
    on the host into each core's dst-sorted CSR slot stream (degree-sorted
    128-node tiles, rectangular padding; pad slots get a_src = -1e30 so that
    exp(lrelu(...)) == 0 exactly).  The host performs no model arithmetic.
  - Kernel-2 (device): per-tile p = exp(lrelu(a_src + a_dst)), q = p * h,
    segment reduction over slots, softmax normalization, bias, and the final
    16->2 linear layer.  All accesses are affine; no indexed DMA.
  (Device-side indexed gathers were measured at ~2us per 128 rows in this
  environment -- ~6.5ms for the 3.3M-edge join -- and the bulk indexed-DMA
  paths (multi-index indirect DMA, dma_gather/scatter_add ucode) are broken
  or unavailable here, so the index-application step lives on the host.)
"""
import os
import sys
import time

for _p in ("/opt/trn_rl_repo", "/root/.axon_site/_ro/trn_rl_repo"):
    if os.path.isdir(_p) and _p not in sys.path:
        sys.path.append(_p)

import numpy as np

N_NODES = 100000
N_CORES = 8
IN_F = 128
HEADS = 8
OUT_C = 2
HC = HEADS * OUT_C          # 16
NEG_SLOPE = 0.2
NODES_PER_CORE = N_NODES // N_CORES   # 12500
P = 128
NT = 100                              # tiles (25 groups of 4)
GRP = 4
NP = NT * P                           # 12800
PAD_ASRC = -1.0e30


# ----------------------------------------------------------------- host prep
def _build_shards(edge_index):
    src = np.asarray(edge_index[0], dtype=np.int64)
    dst = np.asarray(edge_index[1], dtype=np.int64)
    loops = np.arange(N_NODES, dtype=np.int64)
    src = np.concatenate([src, loops])
    dst = np.concatenate([dst, loops])

    core = dst // NODES_PER_CORE
    deg = np.bincount(dst, minlength=N_NODES)

    perms, srcs_by_core, dloc_by_core = [], [], []
    ptab_per_core = np.zeros((N_CORES, NT), np.int64)
    for c in range(N_CORES):
        lo = c * NODES_PER_CORE
        d = deg[lo:lo + NODES_PER_CORE]
        order = np.argsort(-d, kind="stable")
        perm = np.full(NP, -1, np.int64)
        perm[:NODES_PER_CORE] = np.arange(lo, lo + NODES_PER_CORE)[order]
        perms.append(perm)
        dd_pad = np.concatenate([d[order], np.zeros(NP - NODES_PER_CORE, np.int64)])
        ptab_per_core[c] = dd_pad.reshape(NT, P).max(axis=1)
        rank_of_node = np.empty(NODES_PER_CORE, np.int64)
        rank_of_node[order] = np.arange(NODES_PER_CORE)
        m = core == c
        srcs_by_core.append(src[m])
        dloc_by_core.append(rank_of_node[dst[m] - lo])

    ptab = np.maximum(ptab_per_core.max(axis=0), 1)
    ptab = np.repeat(ptab.reshape(NT // GRP, GRP).max(axis=1), GRP)
    S = int((ptab * P).sum())
    tilebase = np.concatenate([[0], np.cumsum(ptab * P)[:-1]])

    slot_srcs = []
    for c in range(N_CORES):
        s = np.full(S, -1, np.int64)
        dloc = dloc_by_core[c]
        esrc = srcs_by_core[c]
        order = np.argsort(dloc, kind="stable")
        dloc_s = dloc[order]
        esrc_s = esrc[order]
        _, cnt = np.unique(dloc_s, return_counts=True)
        j = np.arange(len(dloc_s)) - np.repeat(np.cumsum(cnt) - cnt, cnt)
        ts = dloc_s // P
        ps = dloc_s % P
        s[tilebase[ts] + ps * ptab[ts] + j] = esrc_s
        slot_srcs.append(s)

    return {"perms": perms, "ptab": ptab, "tilebase": tilebase, "S": S,
            "slot_srcs": slot_srcs}


# ------------------------------------------------------------- bass kernels
def _build_kernel1(body_reps=1):
    import concourse.bacc as bacc
    import concourse.tile as tile
    import concourse.mybir as mybir

    nc = bacc.Bacc("TRN2", target_bir_lowering=False, debug=False,
                   enable_asserts=True, num_devices=N_CORES)
    xT = nc.dram_tensor("xT", [P, NP], mybir.dt.float32, kind="ExternalInput").ap()
    Wt = nc.dram_tensor("Wt", [P, HC], mybir.dt.float32, kind="ExternalInput").ap()
    asr = nc.dram_tensor("asr", [P, HC], mybir.dt.float32, kind="ExternalInput").ap()
    adr = nc.dram_tensor("adr", [P, HC], mybir.dt.float32, kind="ExternalInput").ap()
    F = nc.dram_tensor("F", [NP, 32], mybir.dt.float32, kind="ExternalOutput").ap()

    with tile.TileContext(nc) as tc:
        with (
            tc.tile_pool(name="sbuf", bufs=1) as pool,
            tc.tile_pool(name="psum", bufs=4, space="PSUM") as psum,
        ):
            xT_sb = pool.tile([P, NP], mybir.dt.float32)
            W_sb = pool.tile([P, HC], mybir.dt.float32)
            asr_sb = pool.tile([P, HC], mybir.dt.float32)
            adr_sb = pool.tile([P, HC], mybir.dt.float32)
            Fbuf = pool.tile([P, NT, 32], mybir.dt.float32)
            hm = pool.tile([P, NT, HC], mybir.dt.float32)

            nc.sync.dma_start(out=xT_sb[:], in_=xT[:])
            nc.sync.dma_start(out=W_sb[:], in_=Wt[:])
            nc.sync.dma_start(out=asr_sb[:], in_=asr[:])
            nc.sync.dma_start(out=adr_sb[:], in_=adr[:])

            for _rep in range(body_reps):
                for g in range(NT // GRP):
                    ph = psum.tile([P, GRP * HC], mybir.dt.float32, tag="ph")
                    for i in range(GRP):
                        t = g * GRP + i
                        nc.tensor.matmul(out=ph[:, i * HC:(i + 1) * HC],
                                         lhsT=xT_sb[:, t * P:(t + 1) * P],
                                         rhs=W_sb[:], start=True, stop=True)
                    nc.scalar.copy(
                        out=Fbuf[:, g * GRP:(g + 1) * GRP, 0:HC],
                        in_=ph[:].rearrange("p (t f) -> p t f", f=HC))

                hview = Fbuf[:, :, 0:HC]
                for attn, sl in ((asr_sb, slice(16, 24)), (adr_sb, slice(24, 32))):
                    nc.vector.tensor_tensor(
                        out=hm[:], in0=hview,
                        in1=attn[:, None, :].broadcast_to([P, NT, HC]),
                        op=mybir.AluOpType.mult)
                    nc.vector.tensor_reduce(
                        out=Fbuf[:, :, sl],
                        in_=hm[:].rearrange("p t (h c) -> p t h c", c=2),
                        axis=mybir.AxisListType.X, op=mybir.AluOpType.add)

            nc.sync.dma_start(out=F.rearrange("(t p) f -> p t f", p=P), in_=Fbuf[:])
    nc.compile()
    return nc


def _build_kernel2(ptab, tilebase, S, body_reps=1):
    import concourse.bacc as bacc
    import concourse.tile as tile
    import concourse.mybir as mybir

    ptab = [int(v) for v in ptab]
    tilebase = [int(v) for v in tilebase]
    nc = bacc.Bacc("TRN2", target_bir_lowering=False, debug=False,
                   enable_asserts=True, num_devices=N_CORES)
    SF = nc.dram_tensor("SF", [S, 24], mybir.dt.float32, kind="ExternalInput").ap()
    AD = nc.dram_tensor("AD", [NP, HEADS], mybir.dt.float32, kind="ExternalInput").ap()
    brep = nc.dram_tensor("brep", [P, HC], mybir.dt.float32, kind="ExternalInput").ap()
    w0 = nc.dram_tensor("w0", [P, HC], mybir.dt.float32, kind="ExternalInput").ap()
    w1 = nc.dram_tensor("w1", [P, HC], mybir.dt.float32, kind="ExternalInput").ap()
    bfc = nc.dram_tensor("bfc", [P, 2], mybir.dt.float32, kind="ExternalInput").ap()
    OUT = nc.dram_tensor("OUT", [NP, 2], mybir.dt.float32, kind="ExternalOutput").ap()

    pmax = max(ptab)
    with tile.TileContext(nc) as tc:
        with tc.tile_pool(name="sbuf", bufs=1) as cpool, \
             tc.tile_pool(name="feat", bufs=3) as fpool, \
             tc.tile_pool(name="work", bufs=2) as wpool:
            AD_sb = cpool.tile([P, NT, HEADS], mybir.dt.float32)
            brep_sb = cpool.tile([P, HC], mybir.dt.float32)
            w0_sb = cpool.tile([P, HC], mybir.dt.float32)
            w1_sb = cpool.tile([P, HC], mybir.dt.float32)
            bfc_sb = cpool.tile([P, 2], mybir.dt.float32)
            SQ = cpool.tile([P, NT, 24], mybir.dt.float32)
            agg = cpool.tile([P, NT, HC], mybir.dt.float32)
            outb = cpool.tile([P, NT, 2], mybir.dt.float32)

            nc.sync.dma_start(out=AD_sb[:], in_=AD.rearrange("(t p) h -> p t h", p=P))
            nc.sync.dma_start(out=brep_sb[:], in_=brep[:])
            nc.sync.dma_start(out=w0_sb[:], in_=w0[:])
            nc.sync.dma_start(out=w1_sb[:], in_=w1[:])
            nc.sync.dma_start(out=bfc_sb[:], in_=bfc[:])

            for _rep in range(body_reps):
                for g in range(NT // GRP):
                    t0 = g * GRP
                    pt = ptab[t0]            # equal within a group
                    base = tilebase[t0]
                    ns = GRP * pt            # slots per partition in group
                    feat = fpool.tile([P, GRP * pmax * 24], mybir.dt.float32,
                                      tag="feat")
                    nc.sync.dma_start(
                        out=feat[:, :ns * 24].rearrange("p (t x) -> p t x", t=GRP),
                        in_=SF[base:base + GRP * P * pt, :].rearrange(
                            "(t p j) f -> p t (j f)", t=GRP, p=P),
                    )
                    ff = feat[:, :ns * 24]
                    ptile = wpool.tile([P, GRP * pmax * 8], mybir.dt.float32,
                                       tag="pt")
                    # rtile holds [q(16) | p(8)] per slot so ONE reduce covers both
                    rtile = wpool.tile([P, GRP * pmax * 24], mybir.dt.float32,
                                       tag="rt")
                    pp = ptile[:, :ns * 8]
                    rr = rtile[:, :ns * 24]
                    rv = rr.rearrange("p (s f) -> p s f", f=24)
                    # s = a_src + a_dst  (a_dst per tile-in-group)
                    nc.vector.tensor_tensor(
                        out=pp.rearrange("p (t j h) -> p t j h", t=GRP, h=8),
                        in0=ff.rearrange("p (t j f) -> p t j f", t=GRP, f=24)[
                            :, :, :, 16:24],
                        in1=AD_sb[:, t0:t0 + GRP, None, :].broadcast_to(
                            [P, GRP, pt, HEADS]),
                        op=mybir.AluOpType.add)
                    # v = max(0.2*s, s)
                    nc.vector.scalar_tensor_tensor(
                        out=pp, in0=pp, scalar=NEG_SLOPE, in1=pp,
                        op0=mybir.AluOpType.mult, op1=mybir.AluOpType.max)
                    # p = exp(v), written into rtile[:, :, 16:24]
                    nc.scalar.activation(out=rv[:, :, 16:24],
                                         in_=pp.rearrange("p (s h) -> p s h", h=8),
                                         func=mybir.ActivationFunctionType.Exp)
                    # q = p * h, written into rtile[:, :, 0:16]
                    nc.vector.tensor_tensor(
                        out=rv[:, :, 0:16].rearrange("p s (h c) -> p s h c", c=2),
                        in0=ff.rearrange("p (s f) -> p s f", f=24)[
                            :, :, 0:16].rearrange("p s (h c) -> p s h c", c=2),
                        in1=rv[:, :, 16:24][:, :, :, None].broadcast_to(
                            [P, ns, HEADS, 2]),
                        op=mybir.AluOpType.mult)
                    # one segment reduce over j covering [q|p] for all tiles
                    nc.vector.tensor_reduce(
                        out=SQ[:, t0:t0 + GRP, :],
                        in_=rr.rearrange("p (t j f) -> p t f j", t=GRP, f=24),
                        axis=mybir.AxisListType.X, op=mybir.AluOpType.add)

            rec = cpool.tile([P, NT, HEADS], mybir.dt.float32, tag="rec")
            nc.vector.reciprocal(out=rec[:], in_=SQ[:, :, 16:24])
            nc.vector.tensor_tensor(
                out=agg[:].rearrange("p t (h c) -> p t h c", c=2),
                in0=SQ[:, :, 0:16].rearrange("p t (h c) -> p t h c", c=2),
                in1=rec[:, :, :, None].broadcast_to([P, NT, HEADS, 2]),
                op=mybir.AluOpType.mult)
            nc.vector.tensor_tensor(
                out=agg[:], in0=agg[:],
                in1=brep_sb[:, None, :].broadcast_to([P, NT, HC]),
                op=mybir.AluOpType.add)
            tmp = cpool.tile([P, NT, HC], mybir.dt.float32, tag="tmp")
            for wsb, col in ((w0_sb, 0), (w1_sb, 1)):
                nc.vector.tensor_tensor(
                    out=tmp[:], in0=agg[:],
                    in1=wsb[:, None, :].broadcast_to([P, NT, HC]),
                    op=mybir.AluOpType.mult)
                nc.vector.tensor_reduce(out=outb[:, :, col], in_=tmp[:],
                                        axis=mybir.AxisListType.X,
                                        op=mybir.AluOpType.add)
            nc.vector.tensor_tensor(
                out=outb[:], in0=outb[:],
                in1=bfc_sb[:, None, :].broadcast_to([P, NT, 2]),
                op=mybir.AluOpType.add)
            nc.sync.dma_start(out=OUT.rearrange("(t p) c -> p t c", p=P), in_=outb[:])
    nc.compile()
    return nc


# ------------------------------------------------------------------ runner
class _Runner:
    """Reusable jitted shard_map executor for a compiled Bacc kernel."""

    def __init__(self, nc, in_maps):
        import jax
        from jax.sharding import Mesh, PartitionSpec, NamedSharding
        from jax.experimental.shard_map import shard_map
        from concourse import bass2jax, mybir

        bass2jax.install_neuronx_cc_hook()
        partition_name = (nc.partition_id_tensor.name
                          if nc.partition_id_tensor else None)
        in_names, out_names, out_avals, zero_outs = [], [], [], []
        for alloc in nc.m.functions[0].allocations:
            if not isinstance(alloc, mybir.MemoryLocationSet):
                continue
            name = alloc.memorylocations[0].name
            if alloc.kind == "ExternalInput":
                if name != partition_name:
                    in_names.append(name)
            elif alloc.kind == "ExternalOutput":
                shape = tuple(alloc.tensor_shape)
                dtype = mybir.dt.np(alloc.dtype)
                out_names.append(name)
                out_avals.append(jax.core.ShapedArray(shape, dtype))
                zero_outs.append(np.zeros(shape, dtype))
        n_params = len(in_names)
        all_in = list(in_names) + list(out_names)
        if partition_name is not None:
            all_in.append(partition_name)

        def _body(*args):
            operands = list(args)
            if partition_name is not None:
                operands.append(bass2jax.partition_id_tensor())
            return tuple(bass2jax._bass_exec_p.bind(
                *operands, out_avals=tuple(out_avals), in_names=tuple(all_in),
                out_names=tuple(out_names), lowering_input_output_aliases=(),
                sim_require_finite=True, sim_require_nnan=True, nc=nc))

        devices = jax.devices()[:N_CORES]
        mesh = Mesh(np.asarray(devices), ("core",))
        specs = (PartitionSpec("core"),)
        self._fn = jax.jit(
            shard_map(_body, mesh=mesh,
                      in_specs=specs * (n_params + len(out_avals)),
                      out_specs=specs * len(out_avals), check_rep=False),
            keep_unused=True)
        per_core = [[np.asarray(m[name]) for name in in_names] for m in in_maps]
        concat_in = [np.concatenate([per_core[c][i] for c in range(N_CORES)], axis=0)
                     for i in range(n_params)]
        concat_zero = [np.zeros((N_CORES * z.shape[0], *z.shape[1:]), z.dtype)
                       for z in zero_outs]
        sh = NamedSharding(mesh, PartitionSpec("core"))
        self._args = [jax.device_put(a, sh) for a in concat_in + concat_zero]
        self._out_names = out_names
        self._out_avals = out_avals
        self._jax = jax

    def run(self):
        outs = self._fn(*self._args)
        return [
            {name: np.asarray(outs[i]).reshape(N_CORES, *self._out_avals[i].shape)[c]
             for i, name in enumerate(self._out_names)}
            for c in range(N_CORES)
        ]

    def time(self, iters=8, warmup=2):
        for _ in range(warmup):
            self._jax.block_until_ready(self._fn(*self._args))
        walls = []
        for _ in range(iters):
            t0 = time.perf_counter()
            self._jax.block_until_ready(self._fn(*self._args))
            walls.append(time.perf_counter() - t0)
        return min(walls)


# ------------------------------------------------------------------- kernel
def kernel(**inputs):
    x = np.asarray(inputs["x"], np.float32)
    edge_index = np.asarray(inputs["edge_index"])
    W = np.asarray(inputs["W"], np.float32)
    att_src = np.asarray(inputs["att_src"], np.float32)
    att_dst = np.asarray(inputs["att_dst"], np.float32)
    bias_gat = np.asarray(inputs["bias_gat"], np.float32)
    W_fc = np.asarray(inputs["W_fc"], np.float32)
    b_fc = np.asarray(inputs["b_fc"], np.float32)
    # edge_attr intentionally ignored (GATConv built without edge_dim).

    shards = _build_shards(edge_index)

    def _run_retrying(build_nc, maps, attempts=3):
        last = None
        for i in range(attempts):
            try:
                return _Runner(build_nc(), maps).run()
            except Exception as e:  # transient device desync seen on this setup
                last = e
                time.sleep(2.0)
        raise last

    # ---- kernel 1: F = [h | a_src | a_dst], node-sharded
    asr = np.tile(att_src.reshape(1, HC), (P, 1))
    adr = np.tile(att_dst.reshape(1, HC), (P, 1))
    maps1 = []
    for c in range(N_CORES):
        xT = np.zeros((P, NP), np.float32)
        xT[:, :NODES_PER_CORE] = x[c * NODES_PER_CORE:(c + 1) * NODES_PER_CORE].T
        maps1.append({"xT": xT, "Wt": W, "asr": asr, "adr": adr})
    res1 = _run_retrying(_build_kernel1, maps1)
    F_full = np.concatenate([res1[c]["F"][:NODES_PER_CORE] for c in range(N_CORES)],
                            axis=0)

    # ---- host shuffle: materialize per-core dst-CSR slot streams
    S = shards["S"]
    brep = np.tile(bias_gat.reshape(1, HC), (P, 1))
    w0 = np.tile(W_fc[:, 0].reshape(1, HC), (P, 1))
    w1 = np.tile(W_fc[:, 1].reshape(1, HC), (P, 1))
    bfcr = np.tile(b_fc.reshape(1, 2), (P, 1))
    maps2 = []
    for c in range(N_CORES):
        ssrc = shards["slot_srcs"][c]
        perm = shards["perms"][c]
        SF = np.zeros((S, 24), np.float32)
        SF[:, 16:24] = PAD_ASRC
        real = ssrc >= 0
        SF[real] = F_full[ssrc[real], 0:24]
        AD = np.zeros((NP, HEADS), np.float32)
        pr = perm >= 0
        AD[pr] = F_full[perm[pr], 24:32]
        maps2.append({"SF": SF, "AD": AD, "brep": brep, "w0": w0, "w1": w1,
                      "bfc": bfcr})

    # ---- kernel 2: edge math + aggregation + head
    res2 = _run_retrying(
        lambda: _build_kernel2(shards["ptab"], shards["tilebase"], S), maps2)

    out = np.zeros((N_NODES, 2), np.float32)
    for c in range(N_CORES):
        perm = shards["perms"][c]
        pr = perm >= 0
        out[perm[pr]] = res2[c]["OUT"][pr]
    return out
